# revision 1
# baseline (speedup 1.0000x reference)
"""CompGCN layer on 8 Trainium2 cores.

Strategy (dst-sharded, scatter via one-hot matmul):
 - Algebra: msg = (x[src] * rel[t]) @ W, scatter-added over dst with norm
   = inv_deg[dst]. W is linear and norm depends only on dst, so:
       emb = [ scatter_add_e( norm_e * x[src_e] * rel[t_e] ) ] @ W
   i.e. one [N,D]@[D,O] matmul after aggregation instead of per-edge matmuls.
 - Self-loop term (x * loop_rel) @ W_loop = x @ (diag(loop_rel) @ W_loop):
   folded into a host-precomputed weight, needs x.T slices only.
 - Bias + 1/3 scale + BatchNorm applied on host (O(N*D) affine, negligible
   vs the O(E*D) gather/scatter work which all happens on device).
 - Sharding: core k owns dst rows [6250k, 6250k+6250). Host buckets edges by
   (core, subwindow of 128 dsts). All scatter-adds are core-local PSUM
   accumulation -> no inter-core all-reduce needed at all.
 - Per 128-edge tile: indirect-DMA gather x[src] and rel[t] rows (bf16),
   DVE: M = X*R and one-hot Q[e,d] = (iota==dstoff)*norm  (is_equal+mult),
   PE: acc[f,d] += M.T @ Q accumulated in PSUM per 128-dst subwindow.
 - Subwindow epilogue: z[o,d] = W_in.T@(acc_f) + W_out.T@(acc_r)
   + W_loop_eff.T@(xT slice), written to a resident [128, 6272] f32 tile.
"""
import os

os.environ.setdefault("JAX_PLATFORMS", "cpu")

import numpy as np
import ml_dtypes

N = 50000
D = 128
O = 128
E2 = 1600000
NCORES = 8
P = 128
NPC = N // NCORES  # 6250 nodes per core
NSUB = (NPC + P - 1) // P  # 49 subwindows of 128 dsts
NPAD = NSUB * P  # 6272

_cache = {}


def _prep(edge_index, edge_type):
    """Bucket edges by (core, subwindow); build per-core meta arrays."""
    E = edge_index.shape[1] // 2
    row = np.asarray(edge_index[0], dtype=np.int64)
    col = np.asarray(edge_index[1], dtype=np.int64)
    typ = np.asarray(edge_type, dtype=np.int64)

    halves = []
    for h in range(2):
        dst = row[h * E:(h + 1) * E]
        src = col[h * E:(h + 1) * E]
        tp = typ[h * E:(h + 1) * E]
        deg = np.bincount(dst, minlength=N).astype(np.float32)
        inv = 1.0 / np.maximum(deg, 1.0)
        norm = inv[dst]
        core = dst // NPC
        local = dst - core * NPC
        sub = local >> 7
        off = (local & 127).astype(np.float32)
        order = np.argsort(core * NSUB + sub, kind="stable")
        halves.append((src[order], tp[order], off[order], norm[order],
                       (core * NSUB + sub)[order]))

    # tile counts per (sub) equalized across cores
    Tdir = []  # [2][NSUB]
    counts = []  # [2][NCORES*NSUB]
    for h in range(2):
        cnt = np.bincount(halves[h][4], minlength=NCORES * NSUB)
        counts.append(cnt)
        c2 = cnt.reshape(NCORES, NSUB)
        Tdir.append(np.maximum((c2 + 127) // 128, 1).max(axis=0).astype(np.int64))

    Ttot = int(Tdir[0].sum() + Tdir[1].sum())
    # slot layout: for sub in range(NSUB): Tf[sub] fwd tiles, then Tr[sub] rev
    slot_base = np.zeros((2, NSUB), dtype=np.int64)
    t = 0
    for s in range(NSUB):
        slot_base[0, s] = t
        t += Tdir[0][s]
        slot_base[1, s] = t
        t += Tdir[1][s]

    src_all = np.zeros((NCORES, P, Ttot), dtype=np.int32)
    typ_all = np.zeros((NCORES, P, Ttot), dtype=np.int32)
    dst_all = np.full((NCORES, P, Ttot), -1000.0, dtype=np.float32)
    nrm_all = np.zeros((NCORES, P, Ttot), dtype=np.float32)

    for h in range(2):
        src, tp, off, norm, key = halves[h]
        starts = np.zeros(NCORES * NSUB + 1, dtype=np.int64)
        np.cumsum(counts[h], out=starts[1:])
        for k in range(NCORES):
            for s in range(NSUB):
                a, b = starts[k * NSUB + s], starts[k * NSUB + s + 1]
                n = b - a
                if n == 0:
                    continue
                base = slot_base[h, s]
                # edge j (0..n-1) -> slot base + j//128, partition j%128
                sl = base + np.arange(n) // 128
                pp = np.arange(n) & 127
                src_all[k, pp, sl] = src[a:b]
                typ_all[k, pp, sl] = tp[a:b]
                dst_all[k, pp, sl] = off[a:b]
                nrm_all[k, pp, sl] = norm[a:b]

    return (Tdir[0].tolist(), Tdir[1].tolist(), Ttot,
            src_all, typ_all, dst_all, nrm_all)


def _build(Tf, Tr, Ttot):
    import concourse.bass as bass
    import concourse.bacc as bacc
    import concourse.mybir as mybir
    import concourse.tile as tile

    nc = bacc.Bacc("TRN2", target_bir_lowering=False, debug=False,
                   num_devices=NCORES)
    bf16 = mybir.dt.bfloat16
    f32 = mybir.dt.float32
    i32 = mybir.dt.int32

    xg = nc.dram_tensor("xg", [N, D], bf16, kind="ExternalInput")
    relg = nc.dram_tensor("relg", [475, D], bf16, kind="ExternalInput")
    xT = nc.dram_tensor("xT", [P, NPAD], f32, kind="ExternalInput")
    w_in = nc.dram_tensor("w_in", [D, O], f32, kind="ExternalInput")
    w_out = nc.dram_tensor("w_out", [D, O], f32, kind="ExternalInput")
    w_loop = nc.dram_tensor("w_loop", [D, O], f32, kind="ExternalInput")
    iota = nc.dram_tensor("iota", [P, P], f32, kind="ExternalInput")
    srcT = nc.dram_tensor("srcT", [P, Ttot], i32, kind="ExternalInput")
    typT = nc.dram_tensor("typT", [P, Ttot], i32, kind="ExternalInput")
    dstT = nc.dram_tensor("dstT", [P, Ttot], f32, kind="ExternalInput")
    nrmT = nc.dram_tensor("nrmT", [P, Ttot], f32, kind="ExternalInput")
    out_fm = nc.dram_tensor("out_fm", [P, NPAD], f32, kind="ExternalOutput")

    with tile.TileContext(nc) as tc:
        with tc.tile_pool(name="meta", bufs=1) as meta, \
             tc.tile_pool(name="edge", bufs=8) as edge, \
             tc.tile_pool(name="sb", bufs=4) as sbp, \
             tc.tile_pool(name="ps", bufs=2, space="PSUM") as psp:

            src_sb = meta.tile([P, Ttot], i32)
            nc.sync.dma_start(out=src_sb[:], in_=srcT[:, :])
            typ_sb = meta.tile([P, Ttot], i32)
            nc.sync.dma_start(out=typ_sb[:], in_=typT[:, :])
            dst_sb = meta.tile([P, Ttot], f32)
            nc.sync.dma_start(out=dst_sb[:], in_=dstT[:, :])
            nrm_sb = meta.tile([P, Ttot], f32)
            nc.sync.dma_start(out=nrm_sb[:], in_=nrmT[:, :])
            iota_sb = meta.tile([P, P], f32)
            nc.sync.dma_start(out=iota_sb[:], in_=iota[:, :])
            wi_sb = meta.tile([P, O], f32)
            nc.sync.dma_start(out=wi_sb[:], in_=w_in[:, :])
            wo_sb = meta.tile([P, O], f32)
            nc.sync.dma_start(out=wo_sb[:], in_=w_out[:, :])
            wl_sb = meta.tile([P, O], f32)
            nc.sync.dma_start(out=wl_sb[:], in_=w_loop[:, :])
            z_sb = meta.tile([P, NPAD], f32)

            t = 0

            def edge_tile(t, acc, first, last):
                xt = edge.tile([P, D], bf16, tag="xt")
                nc.gpsimd.indirect_dma_start(
                    out=xt[:], out_offset=None, in_=xg[:, :],
                    in_offset=bass.IndirectOffsetOnAxis(
                        ap=src_sb[:, t:t + 1], axis=0))
                rt = edge.tile([P, D], bf16, tag="rt")
                nc.gpsimd.indirect_dma_start(
                    out=rt[:], out_offset=None, in_=relg[:, :],
                    in_offset=bass.IndirectOffsetOnAxis(
                        ap=typ_sb[:, t:t + 1], axis=0))
                m = edge.tile([P, D], bf16, tag="m")
                nc.vector.tensor_tensor(out=m[:], in0=xt[:], in1=rt[:],
                                        op=mybir.AluOpType.mult)
                q = edge.tile([P, P], bf16, tag="q")
                nc.vector.tensor_scalar(
                    out=q[:], in0=iota_sb[:],
                    scalar1=dst_sb[:, t:t + 1], scalar2=nrm_sb[:, t:t + 1],
                    op0=mybir.AluOpType.is_equal, op1=mybir.AluOpType.mult)
                nc.tensor.matmul(acc[:], lhsT=m[:], rhs=q[:],
                                 start=first, stop=last)

            for s in range(NSUB):
                acc_f = psp.tile([P, P], f32, tag="accf")
                for j in range(Tf[s]):
                    edge_tile(t, acc_f, j == 0, j == Tf[s] - 1)
                    t += 1
                acc_r = psp.tile([P, P], f32, tag="accr")
                for j in range(Tr[s]):
                    edge_tile(t, acc_r, j == 0, j == Tr[s] - 1)
                    t += 1

                sbf = sbp.tile([P, P], f32, tag="sbf")
                nc.vector.tensor_copy(out=sbf[:], in_=acc_f[:])
                sbr = sbp.tile([P, P], f32, tag="sbr")
                nc.vector.tensor_copy(out=sbr[:], in_=acc_r[:])
                sbl = sbp.tile([P, P], f32, tag="sbl")
                nc.sync.dma_start(out=sbl[:], in_=xT[:, s * P:(s + 1) * P])

                zp = psp.tile([P, P], f32, tag="zp")
                nc.tensor.matmul(zp[:], lhsT=wi_sb[:], rhs=sbf[:],
                                 start=True, stop=False)
                nc.tensor.matmul(zp[:], lhsT=wo_sb[:], rhs=sbr[:],
                                 start=False, stop=False)
                nc.tensor.matmul(zp[:], lhsT=wl_sb[:], rhs=sbl[:],
                                 start=False, stop=True)
                nc.vector.tensor_copy(out=z_sb[:, s * P:(s + 1) * P], in_=zp[:])

            nc.sync.dma_start(out=out_fm[:, :], in_=z_sb[:])
    nc.compile()
    return nc


def kernel(x, rel_embed, edge_index, edge_type, weight_in, weight_out,
           weight_rel, weight_loop, loop_rel, bias, bn_gamma, bn_beta):
    from concourse.bass_utils import run_bass_kernel_spmd

    x = np.asarray(x)
    rel_embed = np.asarray(rel_embed)
    Tf, Tr, Ttot, src_all, typ_all, dst_all, nrm_all = _prep(
        np.asarray(edge_index), np.asarray(edge_type))

    key = ("k", Ttot, tuple(Tf), tuple(Tr))
    if key not in _cache:
        _cache[key] = _build(Tf, Tr, Ttot)
    nc = _cache[key]

    xg = x.astype(ml_dtypes.bfloat16)
    relg = np.concatenate([rel_embed,
                           np.asarray(loop_rel)], axis=0).astype(ml_dtypes.bfloat16)
    w_loop_eff = (np.asarray(loop_rel)[0][:, None]
                  * np.asarray(weight_loop)).astype(np.float32)
    iota_np = np.broadcast_to(np.arange(P, dtype=np.float32), (P, P)).copy()

    in_maps = []
    for k in range(NCORES):
        xs = np.zeros((P, NPAD), dtype=np.float32)
        xs[:, :NPC] = x[k * NPC:(k + 1) * NPC].T
        in_maps.append({
            "xg": xg, "relg": relg, "xT": xs,
            "w_in": np.ascontiguousarray(weight_in, dtype=np.float32),
            "w_out": np.ascontiguousarray(weight_out, dtype=np.float32),
            "w_loop": w_loop_eff, "iota": iota_np,
            "srcT": src_all[k], "typT": typ_all[k],
            "dstT": dst_all[k], "nrmT": nrm_all[k],
        })

    res = run_bass_kernel_spmd(nc, in_maps, core_ids=list(range(NCORES)),
                               trace=bool(os.environ.get("KTRACE")))
    kernel.last_result = res

    z = np.empty((N, O), dtype=np.float32)
    for k in range(NCORES):
        z[k * NPC:(k + 1) * NPC] = res.results[k]["out_fm"][:, :NPC].T

    # host: bias + /3 + training-mode BN (exact reference semantics on z)
    zz = z * (1.0 / 3.0) + np.asarray(bias)[None, :]
    mean = zz.mean(axis=0)
    var = zz.var(axis=0)
    out = ((zz - mean) / np.sqrt(var + 1e-5) * np.asarray(bn_gamma)[None, :]
           + np.asarray(bn_beta)[None, :]).astype(np.float32)

    rel_full = np.concatenate([rel_embed, np.asarray(loop_rel)], axis=0)
    rel_out = (rel_full @ np.asarray(weight_rel))[:-1].astype(np.float32)
    return out, rel_out


# revision 2
# speedup vs baseline: 1.0023x; 1.0023x over previous
"""CompGCN layer on 8 Trainium2 cores.

Strategy (dst-sharded, scatter via one-hot matmul):
 - Algebra: msg = (x[src] * rel[t]) @ W, scatter-added over dst with norm
   = inv_deg[dst]. W is linear and norm depends only on dst, so:
       emb = [ scatter_add_e( norm_e * x[src_e] * rel[t_e] ) ] @ W
   i.e. one [N,D]@[D,O] matmul after aggregation instead of per-edge matmuls.
 - Self-loop term (x * loop_rel) @ W_loop = x @ (diag(loop_rel) @ W_loop):
   folded into a host-precomputed weight, needs x.T slices only.
 - Bias + 1/3 scale + BatchNorm applied on host (O(N*D) affine, negligible
   vs the O(E*D) gather/scatter work which all happens on device).
 - Sharding: core k owns dst rows [6250k, 6250k+6250). Host buckets edges by
   (core, subwindow of 128 dsts). All scatter-adds are core-local PSUM
   accumulation -> no inter-core all-reduce needed at all.
 - Per 128-edge tile: indirect-DMA gather x[src] and rel[t] rows (bf16),
   DVE: M = X*R and one-hot Q[e,d] = (iota==dstoff)*norm  (is_equal+mult),
   PE: acc[f,d] += M.T @ Q accumulated in PSUM per 128-dst subwindow.
 - Subwindow epilogue: z[o,d] = W_in.T@(acc_f) + W_out.T@(acc_r)
   + W_loop_eff.T@(xT slice), written to a resident [128, 6272] f32 tile.
"""
import os

os.environ.setdefault("JAX_PLATFORMS", "cpu")

import numpy as np
import ml_dtypes

N = 50000
D = 128
O = 128
E2 = 1600000
NCORES = 8
P = 128
NPC = N // NCORES  # 6250 nodes per core
NSUB = (NPC + P - 1) // P  # 49 subwindows of 128 dsts
NPAD = NSUB * P  # 6272

_cache = {}


def _prep(edge_index, edge_type):
    """Bucket edges by (core, subwindow); build per-core meta arrays."""
    E = edge_index.shape[1] // 2
    row = np.asarray(edge_index[0], dtype=np.int64)
    col = np.asarray(edge_index[1], dtype=np.int64)
    typ = np.asarray(edge_type, dtype=np.int64)

    halves = []
    for h in range(2):
        dst = row[h * E:(h + 1) * E]
        src = col[h * E:(h + 1) * E]
        tp = typ[h * E:(h + 1) * E]
        deg = np.bincount(dst, minlength=N).astype(np.float32)
        inv = 1.0 / np.maximum(deg, 1.0)
        norm = inv[dst]
        core = dst // NPC
        local = dst - core * NPC
        sub = local >> 7
        off = (local & 127).astype(np.float32)
        order = np.argsort(core * NSUB + sub, kind="stable")
        halves.append((src[order], tp[order], off[order], norm[order],
                       (core * NSUB + sub)[order]))

    # tile counts per (sub) equalized across cores
    Tdir = []  # [2][NSUB]
    counts = []  # [2][NCORES*NSUB]
    for h in range(2):
        cnt = np.bincount(halves[h][4], minlength=NCORES * NSUB)
        counts.append(cnt)
        c2 = cnt.reshape(NCORES, NSUB)
        Tdir.append(np.maximum((c2 + 127) // 128, 1).max(axis=0).astype(np.int64))

    Ttot = int(Tdir[0].sum() + Tdir[1].sum())
    # slot layout: for sub in range(NSUB): Tf[sub] fwd tiles, then Tr[sub] rev
    slot_base = np.zeros((2, NSUB), dtype=np.int64)
    t = 0
    for s in range(NSUB):
        slot_base[0, s] = t
        t += Tdir[0][s]
        slot_base[1, s] = t
        t += Tdir[1][s]

    src_all = np.zeros((NCORES, P, Ttot), dtype=np.int32)
    typ_all = np.zeros((NCORES, P, Ttot), dtype=np.int32)
    dst_all = np.full((NCORES, P, Ttot), -1000.0, dtype=np.float32)
    nrm_all = np.zeros((NCORES, P, Ttot), dtype=np.float32)

    for h in range(2):
        src, tp, off, norm, key = halves[h]
        starts = np.zeros(NCORES * NSUB + 1, dtype=np.int64)
        np.cumsum(counts[h], out=starts[1:])
        for k in range(NCORES):
            for s in range(NSUB):
                a, b = starts[k * NSUB + s], starts[k * NSUB + s + 1]
                n = b - a
                if n == 0:
                    continue
                base = slot_base[h, s]
                # edge j (0..n-1) -> slot base + j//128, partition j%128
                sl = base + np.arange(n) // 128
                pp = np.arange(n) & 127
                src_all[k, pp, sl] = src[a:b]
                typ_all[k, pp, sl] = tp[a:b]
                dst_all[k, pp, sl] = off[a:b]
                nrm_all[k, pp, sl] = norm[a:b]

    return (Tdir[0].tolist(), Tdir[1].tolist(), Ttot,
            src_all, typ_all, dst_all, nrm_all)


def _build(Tf, Tr, Ttot):
    import concourse.bass as bass
    import concourse.bacc as bacc
    import concourse.mybir as mybir
    import concourse.tile as tile

    nc = bacc.Bacc("TRN2", target_bir_lowering=False, debug=False,
                   num_devices=NCORES)
    bf16 = mybir.dt.bfloat16
    f32 = mybir.dt.float32
    i32 = mybir.dt.int32

    xg = nc.dram_tensor("xg", [N, D], bf16, kind="ExternalInput")
    relg = nc.dram_tensor("relg", [475, D], bf16, kind="ExternalInput")
    xT = nc.dram_tensor("xT", [P, NPAD], f32, kind="ExternalInput")
    w_in = nc.dram_tensor("w_in", [D, O], f32, kind="ExternalInput")
    w_out = nc.dram_tensor("w_out", [D, O], f32, kind="ExternalInput")
    w_loop = nc.dram_tensor("w_loop", [D, O], f32, kind="ExternalInput")
    iota = nc.dram_tensor("iota", [P, P], f32, kind="ExternalInput")
    srcT = nc.dram_tensor("srcT", [P, Ttot], i32, kind="ExternalInput")
    typT = nc.dram_tensor("typT", [P, Ttot], i32, kind="ExternalInput")
    dstT = nc.dram_tensor("dstT", [P, Ttot], f32, kind="ExternalInput")
    nrmT = nc.dram_tensor("nrmT", [P, Ttot], f32, kind="ExternalInput")
    out_fm = nc.dram_tensor("out_fm", [P, NPAD], f32, kind="ExternalOutput")

    with tile.TileContext(nc) as tc:
        with tc.tile_pool(name="meta", bufs=1) as meta, \
             tc.tile_pool(name="edge", bufs=16) as edge, \
             tc.tile_pool(name="sb", bufs=6) as sbp, \
             tc.tile_pool(name="ps", bufs=2, space="PSUM") as psp:

            src_sb = meta.tile([P, Ttot], i32)
            nc.sync.dma_start(out=src_sb[:], in_=srcT[:, :])
            typ_sb = meta.tile([P, Ttot], i32)
            nc.sync.dma_start(out=typ_sb[:], in_=typT[:, :])
            dst_sb = meta.tile([P, Ttot], f32)
            nc.sync.dma_start(out=dst_sb[:], in_=dstT[:, :])
            nrm_sb = meta.tile([P, Ttot], f32)
            nc.sync.dma_start(out=nrm_sb[:], in_=nrmT[:, :])
            iota_sb = meta.tile([P, P], f32)
            nc.sync.dma_start(out=iota_sb[:], in_=iota[:, :])
            wi_sb = meta.tile([P, O], f32)
            nc.sync.dma_start(out=wi_sb[:], in_=w_in[:, :])
            wo_sb = meta.tile([P, O], f32)
            nc.sync.dma_start(out=wo_sb[:], in_=w_out[:, :])
            wl_sb = meta.tile([P, O], f32)
            nc.sync.dma_start(out=wl_sb[:], in_=w_loop[:, :])
            z_sb = meta.tile([P, NPAD], f32)

            t = 0

            def edge_tile(t, acc, first, last):
                xt = edge.tile([P, D], bf16, tag="xt")
                nc.gpsimd.indirect_dma_start(
                    out=xt[:], out_offset=None, in_=xg[:, :],
                    in_offset=bass.IndirectOffsetOnAxis(
                        ap=src_sb[:, t:t + 1], axis=0))
                rt = edge.tile([P, D], bf16, tag="rt")
                nc.gpsimd.indirect_dma_start(
                    out=rt[:], out_offset=None, in_=relg[:, :],
                    in_offset=bass.IndirectOffsetOnAxis(
                        ap=typ_sb[:, t:t + 1], axis=0))
                m = edge.tile([P, D], bf16, tag="m")
                nc.vector.tensor_tensor(out=m[:], in0=xt[:], in1=rt[:],
                                        op=mybir.AluOpType.mult)
                q = edge.tile([P, P], bf16, tag="q")
                nc.vector.tensor_scalar(
                    out=q[:], in0=iota_sb[:],
                    scalar1=dst_sb[:, t:t + 1], scalar2=nrm_sb[:, t:t + 1],
                    op0=mybir.AluOpType.is_equal, op1=mybir.AluOpType.mult)
                nc.tensor.matmul(acc[:], lhsT=m[:], rhs=q[:],
                                 start=first, stop=last)

            for s in range(NSUB):
                acc_f = psp.tile([P, P], f32, tag="accf")
                for j in range(Tf[s]):
                    edge_tile(t, acc_f, j == 0, j == Tf[s] - 1)
                    t += 1
                acc_r = psp.tile([P, P], f32, tag="accr")
                for j in range(Tr[s]):
                    edge_tile(t, acc_r, j == 0, j == Tr[s] - 1)
                    t += 1

                sbf = sbp.tile([P, P], f32, tag="sbf")
                nc.vector.tensor_copy(out=sbf[:], in_=acc_f[:])
                sbr = sbp.tile([P, P], f32, tag="sbr")
                nc.vector.tensor_copy(out=sbr[:], in_=acc_r[:])
                sbl = sbp.tile([P, P], f32, tag="sbl")
                nc.sync.dma_start(out=sbl[:], in_=xT[:, s * P:(s + 1) * P])

                zp = psp.tile([P, P], f32, tag="zp")
                nc.tensor.matmul(zp[:], lhsT=wi_sb[:], rhs=sbf[:],
                                 start=True, stop=False)
                nc.tensor.matmul(zp[:], lhsT=wo_sb[:], rhs=sbr[:],
                                 start=False, stop=False)
                nc.tensor.matmul(zp[:], lhsT=wl_sb[:], rhs=sbl[:],
                                 start=False, stop=True)
                nc.vector.tensor_copy(out=z_sb[:, s * P:(s + 1) * P], in_=zp[:])

            nc.sync.dma_start(out=out_fm[:, :], in_=z_sb[:])
    nc.compile()
    return nc


def kernel(x, rel_embed, edge_index, edge_type, weight_in, weight_out,
           weight_rel, weight_loop, loop_rel, bias, bn_gamma, bn_beta):
    from concourse.bass_utils import run_bass_kernel_spmd

    x = np.asarray(x)
    rel_embed = np.asarray(rel_embed)
    Tf, Tr, Ttot, src_all, typ_all, dst_all, nrm_all = _prep(
        np.asarray(edge_index), np.asarray(edge_type))

    key = ("k", Ttot, tuple(Tf), tuple(Tr))
    if key not in _cache:
        _cache[key] = _build(Tf, Tr, Ttot)
    nc = _cache[key]

    xg = x.astype(ml_dtypes.bfloat16)
    relg = np.concatenate([rel_embed,
                           np.asarray(loop_rel)], axis=0).astype(ml_dtypes.bfloat16)
    w_loop_eff = (np.asarray(loop_rel)[0][:, None]
                  * np.asarray(weight_loop)).astype(np.float32)
    iota_np = np.broadcast_to(np.arange(P, dtype=np.float32), (P, P)).copy()

    in_maps = []
    for k in range(NCORES):
        xs = np.zeros((P, NPAD), dtype=np.float32)
        xs[:, :NPC] = x[k * NPC:(k + 1) * NPC].T
        in_maps.append({
            "xg": xg, "relg": relg, "xT": xs,
            "w_in": np.ascontiguousarray(weight_in, dtype=np.float32),
            "w_out": np.ascontiguousarray(weight_out, dtype=np.float32),
            "w_loop": w_loop_eff, "iota": iota_np,
            "srcT": src_all[k], "typT": typ_all[k],
            "dstT": dst_all[k], "nrmT": nrm_all[k],
        })

    res = run_bass_kernel_spmd(nc, in_maps, core_ids=list(range(NCORES)),
                               trace=bool(os.environ.get("KTRACE")))
    kernel.last_result = res

    z = np.empty((N, O), dtype=np.float32)
    for k in range(NCORES):
        z[k * NPC:(k + 1) * NPC] = res.results[k]["out_fm"][:, :NPC].T

    # host: bias + /3 + training-mode BN (exact reference semantics on z)
    zz = z * (1.0 / 3.0) + np.asarray(bias)[None, :]
    mean = zz.mean(axis=0)
    var = zz.var(axis=0)
    out = ((zz - mean) / np.sqrt(var + 1e-5) * np.asarray(bn_gamma)[None, :]
           + np.asarray(bn_beta)[None, :]).astype(np.float32)

    rel_full = np.concatenate([rel_embed, np.asarray(loop_rel)], axis=0)
    rel_out = (rel_full @ np.asarray(weight_rel))[:-1].astype(np.float32)
    return out, rel_out
